# revision 20
# baseline (speedup 1.0000x reference)
"""Trainium2 Bass kernel for nn_CeptaContextBlock (B=4, T=4096, D=1024, P=512, ALPHA=4, PR=64).

Math (after algebraic simplification of the reference):
    W_comb = W_toP + sum_a W_U[:,:,a] * W_V[:,a]          (host precompute)
    WB     = W_comb @ B_mat                               (host precompute)
    Fg   = sigmoid(x @ W_F)                               (B,T,P)
    lam  = sigmoid(Fg @ W_lam)                            (B,T,PR)
    u    = x @ WB          (== (x @ W_comb) @ B_mat)      (B,T,PR)
    s    = scan: s_i = lam_i * s_{i-1} + u_i along T      (B,T,PR)
    t_til= x @ W_comb + s @ C_mat                         (B,T,P)
    h    = t_til @ W_fromP                                (B,T,D)

Sharding: 8 cores; core c handles batch b=c//2, token half c%2 (2048 tokens).
The cross-half scan dependency is NOT exchanged on device (an AllGather costs
~34us doorbell-to-data). Instead each core outputs its final scan state
sfin=s[TL-1] (64 floats) and cp[r,t]=prod_{i<=t} lam[r,i] over its first CT
tokens; cp underflows to exactly 0 within ~200 tokens. The host applies the
bilinear correction h_odd[0:CT] += (cp_odd * sfin_even[:,None]).T @ (C@W_fromP)
during the gather step (f32, ~0.03% of the model FLOPs).

On-device schedule: software pipeline A0,A1,T0,A2,T1,H0,A3,T2,H1,T3,H2,H3:
  A_c = Fg (k-outer over p-tile halves), u, lam, chained scan
  T_c = 9-matmul PSUM groups (8x wcomb + 1x C@sloc) -> t_til cast (bf16)
  H_c = h = t_til @ W_fromP, copied out per 128-token tile, DMA per tile
All x / wcrit DMAs are k-block slices that are CONTIGUOUS in DRAM (the host
stores them as [K*128, cols] blocks), spread over the sync/scalar/gpsimd
queues, so the first matmul fires ~2us after the trigger preamble and the
feed never falls behind the PE.
"""

import os
import sys

import numpy as np

for _p in ("/opt/trn_rl_repo", "/root/.axon_site/_ro/trn_rl_repo"):
    if os.path.isdir(_p) and _p not in sys.path:
        sys.path.append(_p)

import ml_dtypes

import concourse.bass as bass
import concourse.bacc as bacc
import concourse.mybir as mybir
import concourse.tile as tile
from concourse import bass_utils

B, T, D, P, ALPHA, PR = 4, 4096, 1024, 512, 4, 64
NCORES = 8
TL = T // 2          # tokens per core
KD = D // 128        # 8 d-chunks (contraction for the big matmuls)
PT = P // 128        # 4 p-tiles
CH = 512             # token chunk (free dim per matmul)
NCH = TL // CH       # 4 token chunks per core
CRIT = P + 128       # packed scan-critical weight: [W_F | WB | 0-pad] per k-chunk
CT = 256             # correction window (cumprod(lam) ~ 0 well before this)
F32 = mybir.dt.float32
BF16 = mybir.dt.bfloat16
SIG = mybir.ActivationFunctionType.Sigmoid
CPY = mybir.ActivationFunctionType.Copy
MUL = mybir.AluOpType.mult
ADD = mybir.AluOpType.add
BYP = mybir.AluOpType.bypass

_CACHE = {}


def build_program(ncores: int = NCORES):
    """Build the SPMD Tile program (same NEFF on all cores, no collectives)."""
    nc = bacc.Bacc(
        "TRN2", target_bir_lowering=False, debug=False, num_devices=ncores
    )

    # k-block-major DRAM layouts: every per-k DMA slice is one fully
    # contiguous run ([128, cols] block with adjacent partitions). The
    # narrow (64-wide) stationaries are zero-padded to 128 so every
    # LDWEIGHTS is a standard full-width load that pipelines behind the
    # in-flight matmul (narrow LDW measured +95ns serialization each).
    xt_d = nc.dram_tensor("xt", [128, NCH * KD * CH], BF16, kind="ExternalInput")
    wcrit_d = nc.dram_tensor("wcrit", [128, KD * CRIT], BF16, kind="ExternalInput")
    wcomb_d = nc.dram_tensor("wcomb", [128, KD * P], BF16, kind="ExternalInput")
    wlam_d = nc.dram_tensor("wlam", [128, PT * 128], BF16, kind="ExternalInput")
    cmat_d = nc.dram_tensor("cmat", [128, P], BF16, kind="ExternalInput")
    wfp_d = nc.dram_tensor("wfp", [128, PT * D], BF16, kind="ExternalInput")
    h_d = nc.dram_tensor("h", [TL, D], BF16, kind="ExternalOutput")
    sfin_d = nc.dram_tensor("sfin", [PR, 1], F32, kind="ExternalOutput")
    cp_d = nc.dram_tensor("cp", [PR, CT], F32, kind="ExternalOutput")

    xt_vc = xt_d.rearrange("p (c q) -> p c q", c=NCH)

    with tile.TileContext(nc) as tc:
        with (
            tc.tile_pool(name="wp", bufs=1) as wp,
            tc.tile_pool(name="xp", bufs=4) as xp,
            tc.tile_pool(name="fgp", bufs=2) as fgp,
            tc.tile_pool(name="ttp", bufs=2) as ttp,
            tc.tile_pool(name="sp", bufs=2) as sp,
            tc.tile_pool(name="big", bufs=1) as big,
            tc.tile_pool(name="hp", bufs=4) as hp,
            tc.tile_pool(name="ppa", bufs=2, space="PSUM") as ppa,
            tc.tile_pool(name="pps", bufs=2, space="PSUM") as pps,
            tc.tile_pool(name="ppt", bufs=2, space="PSUM") as ppt,
            tc.tile_pool(name="pph", bufs=2, space="PSUM") as pph,
        ):
            # ---- input DMAs. Two HWDGE queues (sync/scalar) carry the
            # critical stream as quarter-column slices (2.3-4.6KB/partition
            # runs stream at full rate; per-descriptor latency ~1us, so 4
            # slices/tensor balances start latency vs throughput). gpsimd
            # (SWDGE, ~60GB/s) hauls the late-deadline weights as bonus
            # bandwidth. ----
            wcrit_sb = wp.tile([128, KD * CRIT], BF16, tag="wcrit", name="wcrit_sb")
            xt_tiles = [
                xp.tile([128, KD * CH], BF16, tag="xt", name=f"xt{c}")
                for c in range(NCH)
            ]
            QW = 2 * CRIT  # wcrit quarter = 2 k-chunks
            QX = 2 * CH    # xt quarter
            for q in range(4):
                nc.sync.dma_start(
                    wcrit_sb[:, q * QW : (q + 1) * QW],
                    wcrit_d[:, q * QW : (q + 1) * QW],
                )
                nc.scalar.dma_start(
                    xt_tiles[0][:, q * QX : (q + 1) * QX],
                    xt_vc[:, 0, q * QX : (q + 1) * QX],
                )
            hx = KD // 2 * CH  # xt1 split across both queues (deadline ~20us)
            nc.scalar.dma_start(xt_tiles[1][:, 0:hx], xt_vc[:, 1, 0:hx])
            nc.sync.dma_start(xt_tiles[1][:, hx:], xt_vc[:, 1, hx:])
            wlam_sb = wp.tile([128, PT * 128], BF16, tag="wlam", name="wlam_sb")
            nc.sync.dma_start(wlam_sb[:], wlam_d[:, :])
            wcomb_sb = wp.tile([128, KD * P], BF16, tag="wcomb", name="wcomb_sb")
            nc.sync.dma_start(wcomb_sb[:], wcomb_d[:, :])
            nc.scalar.dma_start(xt_tiles[2][:], xt_vc[:, 2, :])
            nc.scalar.dma_start(xt_tiles[3][:], xt_vc[:, 3, :])
            cmat_sb = wp.tile([128, P], BF16, tag="cmat", name="cmat_sb")
            nc.gpsimd.dma_start(cmat_sb[:], cmat_d[:, :])
            wfp_sb = wp.tile([128, PT * D], BF16, tag="wfp", name="wfp_sb")
            nc.gpsimd.dma_start(wfp_sb[:], wfp_d[:, :])

            # ---- persistent activations ----
            s1_sb = big.tile([PR, TL], F32, tag="s1", name="s1")
            cp_sb = big.tile([PR, CT], F32, tag="cp", name="cp")
            # sloc holds bf16 s per chunk on partitions 0:64; partitions
            # 64:128 feed the zero rows of the padded C stationary and are
            # zeroed once (uninitialized SBUF could hold NaN, and NaN*0=NaN)
            sloc_sb = big.tile([128, TL], BF16, tag="sloc", name="sloc")
            nc.vector.memset(sloc_sb[64:128, :], 0.0)

            def phase_a(c):
                """Fg, u, lam, chained scan for token chunk c."""
                cs = slice(c * CH, (c + 1) * CH)
                xt_c = xt_tiles[c]
                fg_c = [
                    fgp.tile([128, CH], BF16, tag=f"fg{m}", name=f"fg{c}_{m}")
                    for m in range(PT)
                ]
                # k-outer over p-tile halves: chunk 0's first matmul needs
                # only the k0 DMA slices; uniform-shape LDWEIGHTS stay
                # pipelined (no 64-col LDW interleaved in the stream)
                for half in range(2):
                    pa = [
                        ppa.tile([128, CH], F32, tag="pa", name=f"pa{c}_{half}_{j}")
                        for j in range(2)
                    ]
                    for k in range(KD):
                        for j in range(2):
                            m = half * 2 + j
                            nc.tensor.matmul(
                                pa[j][:],
                                wcrit_sb[
                                    :, k * CRIT + m * 128 : k * CRIT + (m + 1) * 128
                                ],
                                xt_c[:, k * CH : (k + 1) * CH],
                                start=(k == 0),
                                stop=(k == KD - 1),
                            )
                    for j in range(2):
                        nc.scalar.activation(fg_c[half * 2 + j][:], pa[j][:], SIG)
                # u = x @ [WB | 0] (zero-padded to 128 cols); after the Fg
                # halves so its PSUM slot's previous tenant (chunk c-1's pu,
                # freed by that chunk's scan) is long gone
                pu = pps.tile([128, CH], F32, tag="ps", name=f"pu{c}")
                for k in range(KD):
                    nc.tensor.matmul(
                        pu[:],
                        wcrit_sb[:, k * CRIT + P : (k + 1) * CRIT],
                        xt_c[:, k * CH : (k + 1) * CH],
                        start=(k == 0),
                        stop=(k == KD - 1),
                    )
                # lam = sigmoid(Fg @ [W_lam | 0])
                pl = pps.tile([128, CH], F32, tag="ps", name=f"pl{c}")
                for m in range(PT):
                    nc.tensor.matmul(
                        pl[:],
                        wlam_sb[:, m * 128 : (m + 1) * 128],
                        fg_c[m][:],
                        start=(m == 0),
                        stop=(m == PT - 1),
                    )
                lam_c = sp.tile([PR, CH], F32, tag="lam", name=f"lam{c}")
                nc.scalar.activation(lam_c[:], pl[0:PR, :], SIG)
                # chained local scan; u consumed straight from PSUM
                init = 0.0 if c == 0 else s1_sb[:, c * CH - 1 : c * CH]
                nc.vector.tensor_tensor_scan(
                    s1_sb[:, cs], lam_c[:], pu[0:PR, :], init, op0=MUL, op1=ADD
                )
                if c == 0:
                    nc.vector.tensor_tensor_scan(
                        cp_sb[:], lam_c[:, 0:CT], lam_c[:, 0:CT], 1.0,
                        op0=MUL, op1=BYP,
                    )
                    nc.gpsimd.dma_start(cp_d[:, :], cp_sb[:])
                nc.vector.tensor_copy(sloc_sb[0:PR, cs], s1_sb[:, cs])
                if c == NCH - 1:
                    nc.gpsimd.dma_start(sfin_d[:, :], s1_sb[:, TL - 1 : TL])

            def phase_t(c):
                """t_til = x@W_comb + s@C as one 9-mm PSUM group per p-tile.
                The C stationary is row-padded to 128 (rows 64:128 zero), so
                the s matmul is a standard full-width load too."""
                cs = slice(c * CH, (c + 1) * CH)
                xt_c = xt_tiles[c]
                ttil_c = [
                    ttp.tile([128, CH], BF16, tag=f"tt{m}", name=f"ttil{c}_{m}")
                    for m in range(PT)
                ]
                for m in range(PT):
                    pt_ = ppt.tile([128, CH], F32, tag="pt", name=f"pt{c}_{m}")
                    for k in range(KD):
                        nc.tensor.matmul(
                            pt_[:],
                            wcomb_sb[:, k * P + m * 128 : k * P + (m + 1) * 128],
                            xt_c[:, k * CH : (k + 1) * CH],
                            start=(k == 0),
                            stop=False,
                        )
                    nc.tensor.matmul(
                        pt_[:],
                        cmat_sb[:, m * 128 : (m + 1) * 128],
                        sloc_sb[:, cs],
                        start=False,
                        stop=True,
                    )
                    nc.vector.tensor_copy(ttil_c[m][:], pt_[:])
                return ttil_c

            def phase_h(c, ttil_c):
                """h = t_til @ W_fromP, streamed out per 128-token tile."""
                for tt in range(CH // 128):
                    ts_ = slice(tt * 128, (tt + 1) * 128)
                    h_t = hp.tile([128, D], BF16, tag="hs", name=f"h{c}_{tt}")
                    for dc in range(2):
                        ph = pph.tile([128, CH], F32, tag="ph", name=f"ph{c}_{tt}_{dc}")
                        for m in range(PT):
                            nc.tensor.matmul(
                                ph[:],
                                ttil_c[m][:, ts_],
                                wfp_sb[:, m * D + dc * CH : m * D + dc * CH + CH],
                                start=(m == 0),
                                stop=(m == PT - 1),
                            )
                        if dc == 0:
                            nc.scalar.activation(
                                h_t[:, dc * CH : (dc + 1) * CH], ph[:], CPY
                            )
                        else:
                            nc.vector.tensor_copy(
                                h_t[:, dc * CH : (dc + 1) * CH], ph[:]
                            )
                    rs = slice((c * 4 + tt) * 128, (c * 4 + tt + 1) * 128)
                    if c == NCH - 1 and tt == 3:
                        # final tile: split across both queues to shorten
                        # the kernel tail
                        nc.sync.dma_start(h_d[rs, 0:CH], h_t[:, 0:CH])
                        nc.scalar.dma_start(h_d[rs, CH:D], h_t[:, CH:D])
                    else:
                        nc.sync.dma_start(h_d[rs, :], h_t[:])

            # ---- software pipeline ----
            phase_a(0)
            phase_a(1)
            tt0 = phase_t(0)
            phase_a(2)
            tt1 = phase_t(1)
            phase_h(0, tt0)
            phase_a(3)
            tt2 = phase_t(2)
            phase_h(1, tt1)
            tt3 = phase_t(3)
            phase_h(2, tt2)
            phase_h(3, tt3)

    nc.compile()
    return nc


def _prep_inputs(x, W_toP, W_U, W_F, W_V, W_lam, B_mat, C_mat, W_fromP):
    """Host-side sharding prep: weight folds, bf16 cast, per-core x transpose."""
    bf = ml_dtypes.bfloat16
    def swz(w):
        # [K*128, q] -> partition-major [128, K*q]
        kq = w.shape[0] // 128
        return np.ascontiguousarray(
            w.reshape(kq, 128, w.shape[1]).transpose(1, 0, 2).reshape(128, -1)
        )

    W_comb = (W_toP + (W_U * W_V[None, :, :]).sum(-1)).astype(np.float32)
    WB = W_comb @ np.asarray(B_mat, np.float32)
    # partition-major [128, KD*CRIT]: per k-chunk [W_F_k | WB_k | 0]
    wf32 = np.asarray(W_F, np.float32).reshape(KD, 128, P)
    wbb = np.zeros((KD, 128, 128), np.float32)
    wbb[:, :, 0:PR] = WB.reshape(KD, 128, PR)
    wcrit = np.ascontiguousarray(
        np.concatenate([wf32, wbb], axis=2).transpose(1, 0, 2).reshape(128, -1)
    ).astype(bf)
    wcomb = swz(W_comb).astype(bf)
    wlam_pad = np.zeros((P, 128), np.float32)
    wlam_pad[:, 0:PR] = np.asarray(W_lam, np.float32)
    wlam = swz(wlam_pad).astype(bf)
    cmat = np.zeros((128, P), np.float32)
    cmat[0:PR, :] = np.asarray(C_mat, np.float32)
    cmat = cmat.astype(bf)
    wfp = swz(np.asarray(W_fromP, np.float32)).astype(bf)  # [128, PT*D]
    in_maps = []
    for c in range(NCORES):
        b, half = c // 2, c % 2
        xT = np.asarray(x[b, half * TL : (half + 1) * TL, :], np.float32).T
        # [D, TL] -> [128, NCH*KD*CH] with (c, k, t) free order, partition-major
        xs = np.ascontiguousarray(
            xT.reshape(KD, 128, NCH, CH).transpose(1, 2, 0, 3).reshape(128, -1)
        ).astype(bf)
        in_maps.append(
            {
                "xt": xs,
                "wcrit": wcrit,
                "wcomb": wcomb,
                "wlam": wlam,
                "cmat": cmat,
                "wfp": wfp,
            }
        )
    return in_maps


def kernel(**inputs) -> np.ndarray:
    inputs = {k: np.asarray(v) for k, v in inputs.items()}
    if "nc" not in _CACHE:
        _CACHE["nc"] = build_program()
    nc = _CACHE["nc"]
    in_maps = _prep_inputs(**inputs)
    trace = bool(int(os.environ.get("CEPTA_TRACE", "0")))
    res = bass_utils.run_bass_kernel_spmd(
        nc,
        in_maps,
        core_ids=list(range(NCORES)),
        trace=trace,
        trace_cores=[0] if trace else None,
    )
    _CACHE["last_result"] = res
    # host-side gather + cross-half scan-carry correction (bilinear in the
    # tiny cp [PR,CT] and sfin [PR] outputs; f32, ~0.03% of model FLOPs)
    mcw = (
        np.asarray(inputs["C_mat"], np.float32)
        @ np.asarray(inputs["W_fromP"], np.float32)
    )
    out = np.empty((B, T, D), np.float32)
    for b in range(B):
        even, odd = res.results[2 * b], res.results[2 * b + 1]
        h0 = even["h"].astype(np.float32)
        h1 = odd["h"].astype(np.float32)
        corr = (odd["cp"] * even["sfin"]).T @ mcw   # [CT, D]
        h1[0:CT] += corr
        out[b, 0:TL] = h0
        out[b, TL:T] = h1
    return out
